# revision 6
# baseline (speedup 1.0000x reference)
"""Bass/Tile TRN2 kernel for nn_BernoulliMaskedPPCA (loss_fn).

Math (see reference): with m = int(0.15*D) = 117 masked dims from the LAST
permutation only,
    logits = Wm @ z_int.T + bm[:, None]                  (m, L^2)
    log_prob_x = xm @ log_p1 + (1-xm) @ log_p0           (N, L^2)
               = xm @ logits + sum_j log_p0[j, :]         (x is binary)
    loss = -(D / (P*m*N)) * sum_n logsumexp_c(log_w + log_p_z + log_prob_x)

Strategy (data-parallel, per sharding hint):
  - Host: gather xm = x[:, perm[:m]], transpose to (m+1, N) with a ones row
    appended (folds the per-column constant c_row into the GEMM), cast to
    bf16 (exact for binary x), shard along N across 8 cores.
  - Host: LdAug = [logits; c_row] (118 x 400) built in float64, split into
    bf16 hi + lo so 2 accumulating PE matmuls reproduce fp32 accuracy
    (~7e-7 rel err on the final scalar, validated offline).
  - Device per core: 64 row-tiles of 128; per tile 2 bf16 matmuls into one
    PSUM bank; DVE reduce_max(negate=True) batched over 4 banks; ScalarE
    Exp with per-partition bias (-max) and fused accum_out row-sum.
  - Device outputs per core: S (sum of exps) and -max, each (128, 64) f32.
  - Host: lse = ln(S) + max summed in float64, scaled, returned as f32.
"""

import numpy as np
import ml_dtypes

import concourse.bacc as bacc
import concourse.tile as tile
import concourse.mybir as mybir
from concourse.bass_utils import run_bass_kernel_spmd

N_CORES = 8
N_OBS = 65536
D_DIM = 784
M_DIM = 117  # int(784 * 0.15)
K_DIM = M_DIM + 1  # + ones row for the c_row constant
L_BINS = 20
L2 = L_BINS * L_BINS  # 400
N_PERM = 4
ROWS_PER_CORE = N_OBS // N_CORES  # 8192
PART = 128
N_TILES = ROWS_PER_CORE // PART  # 64
BGRP = 4  # PSUM banks per DVE reduce group
N_GRPS = N_TILES // BGRP  # 16

_COMPILED = None
LAST_RESULTS = None


def _compile():
    global _COMPILED
    if _COMPILED is not None:
        return _COMPILED

    nc = bacc.Bacc("TRN2", target_bir_lowering=False, debug=False)
    xmt_d = nc.dram_tensor(
        "xmt", [K_DIM, ROWS_PER_CORE], mybir.dt.bfloat16, kind="ExternalInput"
    ).ap()
    ldhi_d = nc.dram_tensor(
        "ldhi", [K_DIM, L2], mybir.dt.bfloat16, kind="ExternalInput"
    ).ap()
    ldlo_d = nc.dram_tensor(
        "ldlo", [K_DIM, L2], mybir.dt.bfloat16, kind="ExternalInput"
    ).ap()
    s_d = nc.dram_tensor(
        "s_out", [PART, N_TILES], mybir.dt.float32, kind="ExternalOutput"
    ).ap()
    negm_d = nc.dram_tensor(
        "negm_out", [PART, N_TILES], mybir.dt.float32, kind="ExternalOutput"
    ).ap()

    with tile.TileContext(nc) as tc:
        with (
            tc.tile_pool(name="xpool", bufs=1) as xpool,
            tc.tile_pool(name="consts", bufs=1) as consts,
            tc.tile_pool(name="stats", bufs=1) as stats,
            tc.tile_pool(name="psum", bufs=2, space="PSUM") as psum,
        ):
            xmt_sb = xpool.tile([K_DIM, ROWS_PER_CORE], mybir.dt.bfloat16)
            ldhi_sb = consts.tile([K_DIM, L2], mybir.dt.bfloat16)
            ldlo_sb = consts.tile([K_DIM, L2], mybir.dt.bfloat16)
            negm_sb = stats.tile([PART, N_TILES], mybir.dt.float32)
            s_sb = stats.tile([PART, N_TILES], mybir.dt.float32)

            nc.sync.dma_start(out=ldhi_sb, in_=ldhi_d)
            nc.sync.dma_start(out=ldlo_sb, in_=ldlo_d)
            chunk = 2048
            for k in range(ROWS_PER_CORE // chunk):
                sl = slice(k * chunk, (k + 1) * chunk)
                nc.sync.dma_start(out=xmt_sb[:, sl], in_=xmt_d[:, sl])

            # Prime the exp activation table while input DMAs run, so the
            # ~1.3us table load is off the critical path.
            prime = stats.tile([PART, 1], mybir.dt.float32)
            nc.vector.memset(prime, 0.0)
            nc.scalar.activation(
                out=prime, in_=prime, func=mybir.ActivationFunctionType.Exp
            )

            # The exp shift need not be the exact row max: any per-row value
            # within ~80 of it avoids fp32 overflow/underflow, and the shift
            # is added back exactly, so correctness is shift-independent. A
            # strided submax (every 4th grid column, offset 2) is within ~11
            # of the true max on this problem's data (validated offline, with
            # large margin even under re-randomized inputs), and costs 4x
            # less on the (1x-mode-capped) DVE reduce.
            # Software-pipelined emission: group g's DVE row-sum is emitted
            # after group g+1's submax so the DVE never sits waiting on the
            # ACT exps of the group it just reduced (that serialization lets
            # PE go HAM-cold and doubles the modeled kernel time).
            pending_sum = None  # (yp of previous group, its group index)
            for g in range(N_GRPS):
                yp = psum.tile([PART, BGRP, 512], mybir.dt.float32, tag="yp")
                for i in range(BGRP):
                    t = g * BGRP + i
                    lhsT = xmt_sb[:, t * PART : (t + 1) * PART]
                    nc.tensor.matmul(
                        yp[:, i, 0:L2], lhsT, ldhi_sb, start=True, stop=False
                    )
                    nc.tensor.matmul(
                        yp[:, i, 0:L2], lhsT, ldlo_sb, start=False, stop=True
                    )
                nc.vector.reduce_max(
                    out=negm_sb[:, g * BGRP : (g + 1) * BGRP],
                    in_=yp[:, :, 2:L2:4],
                    axis=mybir.AxisListType.X,
                    negate=True,
                )
                if pending_sum is not None:
                    pyp, pg = pending_sum
                    nc.vector.reduce_sum(
                        out=s_sb[:, pg * BGRP + 1 : (pg + 1) * BGRP],
                        in_=pyp[:, 1:BGRP, 0:L2],
                        axis=mybir.AxisListType.X,
                    )
                # exp in place in PSUM (PSUM src/dst has the smaller ScalarE
                # bubble). Row-sums are split to balance engines: tile 0 of
                # each group via the ACT accumulator, tiles 1..3 via one
                # batched DVE reduce over 3 banks.
                for i in range(BGRP):
                    t = g * BGRP + i
                    if i == 0:
                        nc.scalar.activation(
                            out=yp[:, i, 0:L2],
                            in_=yp[:, i, 0:L2],
                            func=mybir.ActivationFunctionType.Exp,
                            bias=negm_sb[:, t : t + 1],
                            scale=1.0,
                            accum_out=s_sb[:, t : t + 1],
                        )
                    else:
                        nc.scalar.activation(
                            out=yp[:, i, 0:L2],
                            in_=yp[:, i, 0:L2],
                            func=mybir.ActivationFunctionType.Exp,
                            bias=negm_sb[:, t : t + 1],
                            scale=1.0,
                        )
                pending_sum = (yp, g)
            pyp, pg = pending_sum
            nc.vector.reduce_sum(
                out=s_sb[:, pg * BGRP + 1 : (pg + 1) * BGRP],
                in_=pyp[:, 1:BGRP, 0:L2],
                axis=mybir.AxisListType.X,
            )

            nc.sync.dma_start(out=s_d, in_=s_sb)
            nc.sync.dma_start(out=negm_d, in_=negm_sb)

    nc.compile()
    _COMPILED = nc
    return nc


def _host_constants(W, b, perms, L):
    """LdAug (K_DIM, L2) float64: rows 0..m-1 = logits, row m = c_row."""
    perm = np.asarray(perms)[-1]
    idx = perm[:M_DIM]
    Wm = np.asarray(W, np.float64)[idx]
    bm = np.asarray(b, np.float64)[idx]

    zx = np.linspace(-5.0, 5.0, L)
    z1, z2 = np.meshgrid(zx, zx, indexing="xy")
    z_int = np.stack([z1.reshape(-1), z2.reshape(-1)], axis=1)  # (L2, 2)
    log_w = 2.0 * np.log(10.0 / L)
    log_p_z = -np.log(2.0 * np.pi) - 0.5 * np.sum(z_int**2, axis=1)

    logits = Wm @ z_int.T + bm[:, None]  # (m, L2)
    log_p0 = -np.logaddexp(0.0, logits)  # log sigmoid(-logits)
    c_row = log_w + log_p_z + log_p0.sum(axis=0)  # (L2,)
    return np.concatenate([logits, c_row[None, :]], axis=0), idx


def kernel(x, W, b, perms, bins):
    global LAST_RESULTS
    L = int(bins)
    assert L == L_BINS

    LdAug, idx = _host_constants(W, b, perms, L)
    hi = LdAug.astype(ml_dtypes.bfloat16)
    lo = (LdAug - hi.astype(np.float64)).astype(ml_dtypes.bfloat16)

    x_np = np.asarray(x, np.float32)
    assert x_np.shape == (N_OBS, D_DIM)
    xmt = np.empty((K_DIM, N_OBS), dtype=ml_dtypes.bfloat16)
    xmt[:M_DIM] = x_np[:, idx].T  # binary -> exact in bf16
    xmt[M_DIM] = 1.0

    nc = _compile()
    in_maps = []
    for c in range(N_CORES):
        shard = np.ascontiguousarray(
            xmt[:, c * ROWS_PER_CORE : (c + 1) * ROWS_PER_CORE]
        )
        in_maps.append({"xmt": shard, "ldhi": hi, "ldlo": lo})

    res = run_bass_kernel_spmd(nc, in_maps, core_ids=list(range(N_CORES)))
    LAST_RESULTS = res

    total = 0.0
    for c in range(N_CORES):
        s = res.results[c]["s_out"].astype(np.float64)
        mx = -res.results[c]["negm_out"].astype(np.float64)
        total += (np.log(s) + mx).sum()

    loss = -(D_DIM * total) / (N_PERM * M_DIM * N_OBS)
    return np.asarray(loss, dtype=np.float32)


# revision 8
# speedup vs baseline: 39.4123x; 39.4123x over previous
"""Bass/Tile TRN2 kernel for nn_BernoulliMaskedPPCA (loss_fn).

Math (see reference): with m = int(0.15*D) = 117 masked dims from the LAST
permutation only,
    logits = Wm @ z_int.T + bm[:, None]                  (m, L^2)
    log_prob_x = xm @ log_p1 + (1-xm) @ log_p0           (N, L^2)
               = xm @ logits + sum_j log_p0[j, :]         (x is binary)
    loss = -(D / (P*m*N)) * sum_n logsumexp_c(log_w + log_p_z + log_prob_x)

Strategy (data-parallel, per sharding hint):
  - Host: gather xm = x[:, perm[:m]], transpose to (m+1, N) with a ones row
    appended (folds the per-column constant c_row into the GEMM), cast to
    bf16 (exact for binary x), shard along N across 8 cores.
  - Host: LdAug = [logits; c_row] (118 x 400) built in float64, split into
    bf16 hi + lo so 2 accumulating PE matmuls reproduce fp32 accuracy
    (~7e-7 rel err on the final scalar, validated offline).
  - Device per core: 64 row-tiles of 128; per tile 2 bf16 matmuls into one
    PSUM bank; strided DVE submax (negate=True) batched over 4 banks gives
    the exp shift; ScalarE Exp in place in PSUM with per-partition bias;
    row-sums split between the ACT accumulator and batched DVE reduces.
  - Device outputs per core: S (sum of exps) and -shift, each (128, 64) f32.
  - Host: lse = ln(S) + shift summed in float64, scaled, returned as f32.
"""

import numpy as np
import ml_dtypes

import concourse.bacc as bacc
import concourse.tile as tile
import concourse.mybir as mybir
from concourse.bass_utils import run_bass_kernel_spmd

N_CORES = 8
N_OBS = 65536
D_DIM = 784
M_DIM = 117  # int(784 * 0.15)
K_DIM = M_DIM + 1  # + ones row for the c_row constant
L_BINS = 20
L2 = L_BINS * L_BINS  # 400
N_PERM = 4
ROWS_PER_CORE = N_OBS // N_CORES  # 8192
PART = 128
N_TILES = ROWS_PER_CORE // PART  # 64
BGRP = 4  # PSUM banks per DVE reduce group
N_GRPS = N_TILES // BGRP  # 16

_COMPILED = None
LAST_RESULTS = None


def _emit_compute(nc, tc, stats, psum, xmt_d, xmt_sb, ldhi_sb, ldlo_sb,
                  negm_sb, s_sb):
    """One full pass: DMA the x shard in, GEMM + shifted-exp row sums."""
    chunk = 2048
    for k in range(ROWS_PER_CORE // chunk):
        sl = slice(k * chunk, (k + 1) * chunk)
        nc.sync.dma_start(out=xmt_sb[:, sl], in_=xmt_d[:, sl])

    # Prime the exp activation table while input DMAs run, so the ~1.3us
    # table load is off the critical path.
    prime = stats.tile([PART, 1], mybir.dt.float32, tag="prime")
    nc.vector.memset(prime, 0.0)
    nc.scalar.activation(
        out=prime, in_=prime, func=mybir.ActivationFunctionType.Exp
    )

    # The exp shift need not be the exact row max: any per-row value within
    # ~80 of it avoids fp32 overflow/underflow, and the shift is added back
    # exactly, so correctness is shift-independent. A strided submax (every
    # 4th grid column, offset 2) is within ~11 of the true max on this
    # problem's data (validated offline, with large margin even under
    # re-randomized inputs), and costs 4x less on the 1x-mode-capped DVE.
    #
    # Software-pipelined emission: group g's DVE row-sum is emitted after
    # group g+1's submax so the DVE never sits waiting on the ACT exps of
    # the group it just reduced (that serialization lets PE go HAM-cold).
    pending_sum = None  # (yp of previous group, its group index)
    for g in range(N_GRPS):
        yp = psum.tile([PART, BGRP, 512], mybir.dt.float32, tag="yp")
        for i in range(BGRP):
            t = g * BGRP + i
            lhsT = xmt_sb[:, t * PART : (t + 1) * PART]
            nc.tensor.matmul(
                yp[:, i, 0:L2], lhsT, ldhi_sb, start=True, stop=False
            )
            nc.tensor.matmul(
                yp[:, i, 0:L2], lhsT, ldlo_sb, start=False, stop=True
            )
        nc.vector.reduce_max(
            out=negm_sb[:, g * BGRP : (g + 1) * BGRP],
            in_=yp[:, :, 2:L2:4],
            axis=mybir.AxisListType.X,
            negate=True,
        )
        if pending_sum is not None:
            pyp, pg = pending_sum
            nc.vector.reduce_sum(
                out=s_sb[:, pg * BGRP + 1 : (pg + 1) * BGRP],
                in_=pyp[:, 1:BGRP, 0:L2],
                axis=mybir.AxisListType.X,
            )
        # exp in place in PSUM (PSUM src/dst has the smaller ScalarE
        # bubble). Row-sums are split to balance engines: tile 0 of each
        # group via the ACT accumulator, tiles 1..3 via one batched DVE
        # reduce over 3 banks.
        for i in range(BGRP):
            t = g * BGRP + i
            if i == 0:
                nc.scalar.activation(
                    out=yp[:, i, 0:L2],
                    in_=yp[:, i, 0:L2],
                    func=mybir.ActivationFunctionType.Exp,
                    bias=negm_sb[:, t : t + 1],
                    scale=1.0,
                    accum_out=s_sb[:, t : t + 1],
                )
            else:
                nc.scalar.activation(
                    out=yp[:, i, 0:L2],
                    in_=yp[:, i, 0:L2],
                    func=mybir.ActivationFunctionType.Exp,
                    bias=negm_sb[:, t : t + 1],
                    scale=1.0,
                )
        pending_sum = (yp, g)
    pyp, pg = pending_sum
    nc.vector.reduce_sum(
        out=s_sb[:, pg * BGRP + 1 : (pg + 1) * BGRP],
        in_=pyp[:, 1:BGRP, 0:L2],
        axis=mybir.AxisListType.X,
    )


def _build_module(reps=1):
    """Build + bacc-compile the module. reps>1 wraps the compute in a
    device-side loop (bench-only: wall-clock slope over the trip count
    cancels the large axon dispatch overhead)."""
    nc = bacc.Bacc("TRN2", target_bir_lowering=False, debug=False)
    xmt_d = nc.dram_tensor(
        "xmt", [K_DIM, ROWS_PER_CORE], mybir.dt.bfloat16, kind="ExternalInput"
    ).ap()
    ldhi_d = nc.dram_tensor(
        "ldhi", [K_DIM, L2], mybir.dt.bfloat16, kind="ExternalInput"
    ).ap()
    ldlo_d = nc.dram_tensor(
        "ldlo", [K_DIM, L2], mybir.dt.bfloat16, kind="ExternalInput"
    ).ap()
    s_d = nc.dram_tensor(
        "s_out", [PART, N_TILES], mybir.dt.float32, kind="ExternalOutput"
    ).ap()
    negm_d = nc.dram_tensor(
        "negm_out", [PART, N_TILES], mybir.dt.float32, kind="ExternalOutput"
    ).ap()

    with tile.TileContext(nc) as tc:
        with (
            tc.tile_pool(name="xpool", bufs=1) as xpool,
            tc.tile_pool(name="consts", bufs=1) as consts,
            tc.tile_pool(name="stats", bufs=1) as stats,
            tc.tile_pool(name="psum", bufs=2, space="PSUM") as psum,
        ):
            xmt_sb = xpool.tile([K_DIM, ROWS_PER_CORE], mybir.dt.bfloat16)
            ldhi_sb = consts.tile([K_DIM, L2], mybir.dt.bfloat16)
            ldlo_sb = consts.tile([K_DIM, L2], mybir.dt.bfloat16)
            negm_sb = stats.tile([PART, N_TILES], mybir.dt.float32)
            s_sb = stats.tile([PART, N_TILES], mybir.dt.float32)

            nc.sync.dma_start(out=ldhi_sb, in_=ldhi_d)
            nc.sync.dma_start(out=ldlo_sb, in_=ldlo_d)

            if reps == 1:
                _emit_compute(nc, tc, stats, psum, xmt_d, xmt_sb,
                              ldhi_sb, ldlo_sb, negm_sb, s_sb)
            else:
                with tc.For_i(0, reps, 1, hint_engines=(mybir.EngineType.PE,)):
                    _emit_compute(nc, tc, stats, psum, xmt_d, xmt_sb,
                                  ldhi_sb, ldlo_sb, negm_sb, s_sb)

            nc.sync.dma_start(out=s_d, in_=s_sb)
            nc.sync.dma_start(out=negm_d, in_=negm_sb)

    nc.compile()
    return nc


def _compile():
    global _COMPILED
    if _COMPILED is None:
        _COMPILED = _build_module(reps=1)
    return _COMPILED


def _host_constants(W, b, perms, L):
    """LdAug (K_DIM, L2) float64: rows 0..m-1 = logits, row m = c_row."""
    perm = np.asarray(perms)[-1]
    idx = perm[:M_DIM]
    Wm = np.asarray(W, np.float64)[idx]
    bm = np.asarray(b, np.float64)[idx]

    zx = np.linspace(-5.0, 5.0, L)
    z1, z2 = np.meshgrid(zx, zx, indexing="xy")
    z_int = np.stack([z1.reshape(-1), z2.reshape(-1)], axis=1)  # (L2, 2)
    log_w = 2.0 * np.log(10.0 / L)
    log_p_z = -np.log(2.0 * np.pi) - 0.5 * np.sum(z_int**2, axis=1)

    logits = Wm @ z_int.T + bm[:, None]  # (m, L2)
    log_p0 = -np.logaddexp(0.0, logits)  # log sigmoid(-logits)
    c_row = log_w + log_p_z + log_p0.sum(axis=0)  # (L2,)
    return np.concatenate([logits, c_row[None, :]], axis=0), idx


def kernel(x, W, b, perms, bins):
    global LAST_RESULTS
    L = int(bins)
    assert L == L_BINS

    LdAug, idx = _host_constants(W, b, perms, L)
    hi = LdAug.astype(ml_dtypes.bfloat16)
    lo = (LdAug - hi.astype(np.float64)).astype(ml_dtypes.bfloat16)

    x_np = np.asarray(x, np.float32)
    assert x_np.shape == (N_OBS, D_DIM)
    xmt = np.empty((K_DIM, N_OBS), dtype=ml_dtypes.bfloat16)
    xmt[:M_DIM] = x_np[:, idx].T  # binary -> exact in bf16
    xmt[M_DIM] = 1.0

    nc = _compile()
    in_maps = []
    for c in range(N_CORES):
        shard = np.ascontiguousarray(
            xmt[:, c * ROWS_PER_CORE : (c + 1) * ROWS_PER_CORE]
        )
        in_maps.append({"xmt": shard, "ldhi": hi, "ldlo": lo})

    res = run_bass_kernel_spmd(nc, in_maps, core_ids=list(range(N_CORES)))
    LAST_RESULTS = res

    total = 0.0
    for c in range(N_CORES):
        s = res.results[c]["s_out"].astype(np.float64)
        mx = -res.results[c]["negm_out"].astype(np.float64)
        total += (np.log(s) + mx).sum()

    loss = -(D_DIM * total) / (N_PERM * M_DIM * N_OBS)
    return np.asarray(loss, dtype=np.float32)


# revision 19
# speedup vs baseline: 59.4945x; 1.5095x over previous
"""Bass/Tile TRN2 kernel for nn_BernoulliMaskedPPCA (loss_fn).

Math (see reference): with m = int(0.15*D) = 117 masked dims from the LAST
permutation only,
    logits = Wm @ z_int.T + bm[:, None]                  (m, L^2)
    log_prob_x = xm @ log_p1 + (1-xm) @ log_p0           (N, L^2)
               = xm @ logits + sum_j log_p0[j, :]         (x is binary)
    loss = -(D / (P*m*N)) * sum_n logsumexp_c(log_w + log_p_z + log_prob_x)

Strategy (data-parallel, per sharding hint):
  - Host: gather xm = x[:, perm[:m]], transpose to (m+1, N) with a ones row
    appended (folds the per-column constant c_row into the GEMM), cast to
    bf16 (exact for binary x), shard along N across 8 cores.
  - Host: LdAug = [logits; c_row] (118 x 400) built in float64, split into
    bf16 hi + lo so 2 accumulating PE matmuls reproduce fp32 accuracy
    (~7e-7 rel err on the final scalar, validated offline).
  - Device per core: 64 row-tiles of 128; per tile 2 bf16 matmuls into one
    PSUM bank; strided DVE submax (negate=True) batched over 4 banks gives
    the exp shift; ScalarE Exp in place in PSUM with per-partition bias;
    row-sums split between the ACT accumulator and batched DVE reduces.
  - Device outputs per core: S (sum of exps) and -shift, each (128, 64) f32.
  - Host: lse = ln(S) + shift summed in float64, scaled, returned as f32.
"""

import numpy as np
import ml_dtypes

import concourse.bacc as bacc
import concourse.tile as tile
import concourse.mybir as mybir
from concourse.bass_utils import run_bass_kernel_spmd

N_CORES = 8
N_OBS = 65536
D_DIM = 784
M_DIM = 117  # int(784 * 0.15)
K_DIM = M_DIM + 1  # + ones row for the c_row constant
L_BINS = 20
L2 = L_BINS * L_BINS  # 400
N_PERM = 4
ROWS_PER_CORE = N_OBS // N_CORES  # 8192
PART = 128
N_TILES = ROWS_PER_CORE // PART  # 64
BGRP = 2  # PSUM banks per group (bufs=4 -> 4 groups in flight)
N_GRPS = N_TILES // BGRP  # 32

_COMPILED = None
LAST_RESULTS = None


def _emit_compute(nc, tc, stats, psum, xmt_d, xmt_sb, ldhi_sb, ldlo_sb,
                  negm_sb, s_sb, do_dve=True, do_act=True, act_accum=True):
    """One full pass: DMA the x shard in, GEMM + shifted-exp row sums.

    do_dve/do_act/act_accum are benchmark-only ablation switches
    (numerically wrong when False; used to attribute HW time per engine)."""
    chunk = 2048
    for k in range(ROWS_PER_CORE // chunk):
        sl = slice(k * chunk, (k + 1) * chunk)
        nc.sync.dma_start(out=xmt_sb[:, sl], in_=xmt_d[:, sl])

    # Prime the exp activation table while input DMAs run, so the ~1.3us
    # table load is off the critical path.
    if do_act:
        prime = stats.tile([PART, 1], mybir.dt.float32, tag="prime")
        nc.vector.memset(prime, 0.0)
        nc.scalar.activation(
            out=prime, in_=prime, func=mybir.ActivationFunctionType.Exp
        )

    # The exp shift need not be the exact row max: any per-row value within
    # ~80 of it avoids fp32 overflow/underflow, and the shift is added back
    # exactly, so correctness is shift-independent. A strided submax (every
    # 4th grid column, offset 2) is within ~11 of the true max on this
    # problem's data (validated offline, with large margin even under
    # re-randomized inputs), and costs 4x less on the 1x-mode-capped DVE.
    #
    # Software-pipelined emission: group g's DVE row-sum is emitted after
    # group g+1's submax so the DVE never sits waiting on the ACT exps of
    # the group it just reduced. An accumulator tile every other group
    # (f=1/4 of tiles) offloads some row-sums from DVE to ScalarE.
    def group_lo_bank(g):
        return 1 if (act_accum and g % 2 == 0) else 0

    def emit_sum(pyp, pg):
        lb = group_lo_bank(pg)
        nc.vector.reduce_sum(
            out=s_sb[:, pg * BGRP + lb : (pg + 1) * BGRP],
            in_=pyp[:, lb:BGRP, 0:L2],
            axis=mybir.AxisListType.X,
        )

    pending_sum = None  # (yp of previous group, its group index)
    for g in range(N_GRPS):
        yp = psum.tile([PART, BGRP, 512], mybir.dt.float32, tag="yp")
        for i in range(BGRP):
            t = g * BGRP + i
            lhsT = xmt_sb[:, t * PART : (t + 1) * PART]
            nc.tensor.matmul(
                yp[:, i, 0:L2], lhsT, ldhi_sb, start=True, stop=False
            )
            nc.tensor.matmul(
                yp[:, i, 0:L2], lhsT, ldlo_sb, start=False, stop=True
            )
        if do_dve:
            nc.vector.reduce_max(
                out=negm_sb[:, g * BGRP : (g + 1) * BGRP],
                in_=yp[:, :, 2:L2:4],
                axis=mybir.AxisListType.X,
                negate=True,
            )
            if pending_sum is not None:
                emit_sum(*pending_sum)
        # exp in place in PSUM (PSUM src/dst has the smaller ScalarE bubble)
        if do_act:
            for i in range(BGRP):
                t = g * BGRP + i
                if i == 0 and act_accum and g % 2 == 0:
                    nc.scalar.activation(
                        out=yp[:, i, 0:L2],
                        in_=yp[:, i, 0:L2],
                        func=mybir.ActivationFunctionType.Exp,
                        bias=negm_sb[:, t : t + 1],
                        scale=1.0,
                        accum_out=s_sb[:, t : t + 1],
                    )
                else:
                    nc.scalar.activation(
                        out=yp[:, i, 0:L2],
                        in_=yp[:, i, 0:L2],
                        func=mybir.ActivationFunctionType.Exp,
                        bias=negm_sb[:, t : t + 1],
                        scale=1.0,
                    )
        pending_sum = (yp, g)
    if do_dve:
        emit_sum(*pending_sum)


def _build_module(reps=1, do_dve=True, do_act=True, act_accum=True):
    """Build + bacc-compile the module. reps>1 wraps the compute in a
    device-side loop (bench-only: wall-clock slope over the trip count
    cancels the large axon dispatch overhead)."""
    nc = bacc.Bacc("TRN2", target_bir_lowering=False, debug=False)
    xmt_d = nc.dram_tensor(
        "xmt", [K_DIM, ROWS_PER_CORE], mybir.dt.bfloat16, kind="ExternalInput"
    ).ap()
    ldhi_d = nc.dram_tensor(
        "ldhi", [K_DIM, L2], mybir.dt.bfloat16, kind="ExternalInput"
    ).ap()
    ldlo_d = nc.dram_tensor(
        "ldlo", [K_DIM, L2], mybir.dt.bfloat16, kind="ExternalInput"
    ).ap()
    s_d = nc.dram_tensor(
        "s_out", [PART, N_TILES], mybir.dt.float32, kind="ExternalOutput"
    ).ap()
    negm_d = nc.dram_tensor(
        "negm_out", [PART, N_TILES], mybir.dt.float32, kind="ExternalOutput"
    ).ap()

    with tile.TileContext(nc) as tc:
        with (
            tc.tile_pool(name="xpool", bufs=1) as xpool,
            tc.tile_pool(name="consts", bufs=1) as consts,
            tc.tile_pool(name="stats", bufs=1) as stats,
            tc.tile_pool(name="psum", bufs=4, space="PSUM") as psum,
        ):
            xmt_sb = xpool.tile([K_DIM, ROWS_PER_CORE], mybir.dt.bfloat16)
            ldhi_sb = consts.tile([K_DIM, L2], mybir.dt.bfloat16)
            ldlo_sb = consts.tile([K_DIM, L2], mybir.dt.bfloat16)
            negm_sb = stats.tile([PART, N_TILES], mybir.dt.float32)
            s_sb = stats.tile([PART, N_TILES], mybir.dt.float32)

            nc.sync.dma_start(out=ldhi_sb, in_=ldhi_d)
            nc.sync.dma_start(out=ldlo_sb, in_=ldlo_d)

            if not (do_dve and do_act):
                # ablation variants leave parts of the outputs unwritten;
                # initialize so the output DMAs have allocated sources
                nc.gpsimd.memset(s_sb, 1.0)
                nc.gpsimd.memset(negm_sb, 0.0)

            if reps == 1:
                _emit_compute(nc, tc, stats, psum, xmt_d, xmt_sb,
                              ldhi_sb, ldlo_sb, negm_sb, s_sb,
                              do_dve=do_dve, do_act=do_act, act_accum=act_accum)
            else:
                with tc.For_i(0, reps, 1, hint_engines=(mybir.EngineType.PE,)):
                    _emit_compute(nc, tc, stats, psum, xmt_d, xmt_sb,
                                  ldhi_sb, ldlo_sb, negm_sb, s_sb,
                                  do_dve=do_dve, do_act=do_act,
                                  act_accum=act_accum)

            nc.sync.dma_start(out=s_d, in_=s_sb)
            nc.sync.dma_start(out=negm_d, in_=negm_sb)

    nc.compile()
    return nc


def _compile():
    global _COMPILED
    if _COMPILED is None:
        _COMPILED = _build_module(reps=1)
    return _COMPILED


def _host_constants(W, b, perms, L):
    """LdAug (K_DIM, L2) float64: rows 0..m-1 = logits, row m = c_row."""
    perm = np.asarray(perms)[-1]
    idx = perm[:M_DIM]
    Wm = np.asarray(W, np.float64)[idx]
    bm = np.asarray(b, np.float64)[idx]

    zx = np.linspace(-5.0, 5.0, L)
    z1, z2 = np.meshgrid(zx, zx, indexing="xy")
    z_int = np.stack([z1.reshape(-1), z2.reshape(-1)], axis=1)  # (L2, 2)
    log_w = 2.0 * np.log(10.0 / L)
    log_p_z = -np.log(2.0 * np.pi) - 0.5 * np.sum(z_int**2, axis=1)

    logits = Wm @ z_int.T + bm[:, None]  # (m, L2)
    log_p0 = -np.logaddexp(0.0, logits)  # log sigmoid(-logits)
    c_row = log_w + log_p_z + log_p0.sum(axis=0)  # (L2,)
    return np.concatenate([logits, c_row[None, :]], axis=0), idx


def kernel(x, W, b, perms, bins):
    global LAST_RESULTS
    L = int(bins)
    assert L == L_BINS

    LdAug, idx = _host_constants(W, b, perms, L)
    hi = LdAug.astype(ml_dtypes.bfloat16)
    lo = (LdAug - hi.astype(np.float64)).astype(ml_dtypes.bfloat16)

    x_np = np.asarray(x, np.float32)
    assert x_np.shape == (N_OBS, D_DIM)
    xmt = np.empty((K_DIM, N_OBS), dtype=ml_dtypes.bfloat16)
    xmt[:M_DIM] = x_np[:, idx].T  # binary -> exact in bf16
    xmt[M_DIM] = 1.0

    nc = _compile()
    in_maps = []
    for c in range(N_CORES):
        shard = np.ascontiguousarray(
            xmt[:, c * ROWS_PER_CORE : (c + 1) * ROWS_PER_CORE]
        )
        in_maps.append({"xmt": shard, "ldhi": hi, "ldlo": lo})

    res = run_bass_kernel_spmd(nc, in_maps, core_ids=list(range(N_CORES)))
    LAST_RESULTS = res

    total = 0.0
    for c in range(N_CORES):
        s = res.results[c]["s_out"].astype(np.float64)
        mx = -res.results[c]["negm_out"].astype(np.float64)
        total += (np.log(s) + mx).sum()

    loss = -(D_DIM * total) / (N_PERM * M_DIM * N_OBS)
    return np.asarray(loss, dtype=np.float32)
